# revision 60
# baseline (speedup 1.0000x reference)
"""HMM forward log-likelihood (CgpHmmLayer) on 8 TRN2 NeuronCores.

Data-parallel over batch: 128 sequences -> 16 per core. Each core runs the
full T=4096 alpha scan for its shard with A/B/I kernels replicated; the
per-core [16] loglik outputs are concatenated on the host.

Layout: alpha is kept transposed ([state, seq]) with the 308 states split
into 3 partition blocks (128/128/52, padded), one SBUF tile per block so
dependency tracking is per-slice. Per scan step: 9 matmuls (3 k-chunks x 3
m-blocks; the A-tile is the stationary operand, alpha the N=16 moving one)
accumulate each m-block into its own PSUM bank (start=True zeroes a whole
bank, so concurrently-accumulating slices must not share one); 3 DVE
multiplies then apply the per-(state,seq) emission and write the next alpha
slices, letting step t+1's k-chunk-j matmuls start as soon as slice j is
ready. Emissions for window w+1 are built during window w via one-hot
matmuls (obs -> onehot[6,cols] -> Bm.T @ onehot). A factor of 6 is folded
into the emission matrix so the running scale stays O(1) (uniform obs make
E[z]=1/6 exactly); an exact renormalization (sum via ones-matmul, ln
accumulated into the telescoped loglik, reciprocal broadcast via a K=1
matmul) runs every 4 windows. All softmaxes run on device.

Measured on trn2.8x1: 3.13 ms HW exec, rel err 6.9e-6 vs the fp32 jax
reference (~700 ns/scan-step; critical path = matmul drain 190 + sem 38 +
DVE emission-mult 175 + sem 54 + ~240 of data/weight-paced streaming, plus
~80 ns/step of emission-precompute amortization). The kernel re-executes
itself in a PYTHONHASHSEED=0 subprocess because Tile's schedule is
hash-order sensitive (3.13 vs 3.75 ms bimodal otherwise).
"""

import sys
import types

sys.path.insert(0, "/opt/trn_rl_repo")

import math

import numpy as np

# If BASS_TRACE is set but this image's antenv lacks axon_hooks, the trace
# path of run_bass_kernel_spmd would die on import. Pre-install a stub that
# reports "no hook" so tracing degrades gracefully instead. A real hook
# installed earlier (e.g. by test.py) is left untouched.
try:
    from antenv.axon_hooks import get_axon_ntff_profile_hook  # noqa: F401
except ImportError:
    _mod = types.ModuleType("antenv.axon_hooks")
    _mod._hook = None
    _mod.get_axon_ntff_profile_hook = lambda: _mod._hook
    _mod.set_axon_ntff_profile_hook = lambda h: setattr(_mod, "_hook", h)
    sys.modules["antenv.axon_hooks"] = _mod

import concourse.bass as bass
import concourse.bacc as bacc
import concourse.mybir as mybir
from concourse.alu_op_type import AluOpType
from concourse.bass_utils import run_bass_kernel_spmd
from concourse.tile import TileContext

F32 = mybir.dt.float32
BF16 = mybir.dt.bfloat16
I32 = mybir.dt.int32

S, EM = 308, 6
NB = 16          # sequences per core
NCORES = 8
CSCALE = 6.0     # folded into emission probs to keep running scale ~O(1)


def build(T=4096, WIN=256):
    assert T % WIN == 0
    nwin = T // WIN
    cols = WIN * NB          # onehot columns per window
    assert cols % 512 == 0
    nh = cols // 512

    nc = bacc.Bacc("TRN2", target_bir_lowering=False, debug=False)
    obsT = nc.declare_dram_parameter("obsT", [T, NB], I32, isOutput=False)
    A_k = nc.declare_dram_parameter("A_kern", [S, S], F32, isOutput=False)
    B_k = nc.declare_dram_parameter("B_kern", [S, EM], F32, isOutput=False)
    I_k = nc.declare_dram_parameter("I_kern", [1, S], F32, isOutput=False)
    out_d = nc.declare_dram_parameter("out", [1, NB], F32, isOutput=True)
    scrB = nc.dram_tensor("scrB", [S, EM], BF16)
    scrI = nc.dram_tensor("scrI", [1, S], F32)

    PBLK = [128, 128, 52]   # rows of each state block

    with TileContext(nc) as tc:
        import contextlib
        ctx = contextlib.ExitStack()
        with ctx:
            const = ctx.enter_context(tc.tile_pool(name="const", bufs=1))
            work = ctx.enter_context(tc.tile_pool(name="work", bufs=2))
            obs_pool = ctx.enter_context(tc.tile_pool(name="obsp", bufs=2))
            oh_pool = ctx.enter_context(tc.tile_pool(name="ohp", bufs=9))
            em_pools = [
                ctx.enter_context(tc.tile_pool(name=f"em{b}", bufs=2))
                for b in range(3)
            ]
            alpha_pool = [
                ctx.enter_context(tc.tile_pool(name=f"alphap{b}", bufs=1))
                for b in range(3)
            ]
            small = ctx.enter_context(tc.tile_pool(name="small", bufs=2))
            # PSUM budget (8 banks): two 3-bank scan supertiles (each scan
            # slice accumulates in its own bank since start=True zeroes the
            # whole bank; one DVE mult then reads all three via a strided
            # AP), emission+onehot share one bank, normalization one.
            ps_sl = [
                ctx.enter_context(
                    tc.tile_pool(name=f"ps_s{b}", bufs=1, space="PSUM"))
                for b in range(3)
            ]
            ps_e = ctx.enter_context(tc.tile_pool(name="ps_e", bufs=2, space="PSUM"))

            # ---------------- constants / preprocessing ----------------
            # A row-softmax -> 3 k-blocks [128, 384] bf16 (zero padded)
            Abf = []
            for k in range(3):
                P = PBLK[k]
                araw = work.tile([128, S], F32, tag="araw")
                nc.sync.dma_start(araw[:P, :], A_k[128 * k:128 * k + P, :])
                negmax = work.tile([128, 1], F32, tag="negmax")
                nc.vector.tensor_reduce(
                    negmax[:P], araw[:P, :], axis=mybir.AxisListType.X,
                    op=AluOpType.max, negate=True)
                ex = work.tile([128, S], F32, tag="ex")
                nc.scalar.activation(
                    ex[:P, :], araw[:P, :], mybir.ActivationFunctionType.Exp,
                    bias=negmax[:P])
                ssum = work.tile([128, 1], F32, tag="ssum")
                nc.vector.tensor_reduce(
                    ssum[:P], ex[:P, :], axis=mybir.AxisListType.X,
                    op=AluOpType.add)
                rs = work.tile([128, 1], F32, tag="rs")
                nc.vector.reciprocal(rs[:P], ssum[:P])
                At = const.tile([128, 384], BF16, tag=f"A{k}")
                nc.vector.memset(At[:, :], 0.0)
                nc.vector.tensor_scalar_mul(At[:P, 0:S], ex[:P, :], rs[:P])
                Abf.append(At)

            # B row-softmax * CSCALE -> scratch DRAM -> transposed BmT [6,384]
            for k in range(3):
                P = PBLK[k]
                braw = work.tile([128, EM], F32, tag="braw")
                nc.sync.dma_start(braw[:P, :], B_k[128 * k:128 * k + P, :])
                negmax = work.tile([128, 1], F32, tag="negmax")
                nc.vector.tensor_reduce(
                    negmax[:P], braw[:P, :], axis=mybir.AxisListType.X,
                    op=AluOpType.max, negate=True)
                ex = work.tile([128, EM], F32, tag="exb")
                nc.scalar.activation(
                    ex[:P, :], braw[:P, :], mybir.ActivationFunctionType.Exp,
                    bias=negmax[:P])
                ssum = work.tile([128, 1], F32, tag="ssum")
                nc.vector.tensor_reduce(
                    ssum[:P], ex[:P, :], axis=mybir.AxisListType.X,
                    op=AluOpType.add)
                rs = work.tile([128, 1], F32, tag="rs")
                nc.vector.reciprocal(rs[:P], ssum[:P])
                bmn = work.tile([128, EM], BF16, tag="bmn")
                nc.vector.tensor_scalar(
                    bmn[:P, :], ex[:P, :], rs[:P], CSCALE,
                    op0=AluOpType.mult, op1=AluOpType.mult)
                nc.sync.dma_start(scrB[128 * k:128 * k + P, :], bmn[:P, :])
            BmT = const.tile([6, 384], BF16, tag="BmT")
            nc.vector.memset(BmT[:, :], 0.0)
            for k in range(3):
                P = PBLK[k]
                nc.sync.dma_start(
                    BmT[:, 128 * k:128 * k + P],
                    scrB[128 * k:128 * k + P, :].rearrange("a b -> b a"))

            # I softmax -> scratch -> I_col [128, 3] f32 (zero padded)
            iraw = work.tile([1, S], F32, tag="iraw")
            nc.sync.dma_start(iraw[:1, :], I_k[0:1, :])
            inegmax = work.tile([1, 1], F32, tag="inegmax")
            nc.vector.tensor_reduce(
                inegmax[:1], iraw[:1, :], axis=mybir.AxisListType.X,
                op=AluOpType.max, negate=True)
            iex = work.tile([1, S], F32, tag="iex")
            nc.scalar.activation(
                iex[:1, :], iraw[:1, :], mybir.ActivationFunctionType.Exp,
                bias=inegmax[:1])
            issum = work.tile([1, 1], F32, tag="issum")
            nc.vector.tensor_reduce(
                issum[:1], iex[:1, :], axis=mybir.AxisListType.X,
                op=AluOpType.add)
            irs = work.tile([1, 1], F32, tag="irs")
            nc.vector.reciprocal(irs[:1], issum[:1])
            ism = work.tile([1, S], F32, tag="ism")
            nc.vector.tensor_scalar_mul(ism[:1, :], iex[:1, :], irs[:1])
            nc.sync.dma_start(scrI[0:1, :], ism[:1, :])
            I_col = const.tile([128, 3], F32, tag="I_col")
            nc.vector.memset(I_col[:, :], 0.0)
            for k in range(3):
                P = PBLK[k]
                nc.sync.dma_start(
                    I_col[:P, k:k + 1],
                    scrI[0:1, 128 * k:128 * k + P].rearrange("a b -> b a"))

            ones6 = const.tile([1, 6], BF16, tag="ones6")
            nc.vector.memset(ones6[:, :], 1.0)
            onesz = const.tile([128, 1], BF16, tag="onesz")
            nc.vector.memset(onesz[:, :], 1.0)
            onesb = const.tile([1, 128], F32, tag="onesb")
            nc.vector.memset(onesb[:, :], 1.0)
            iota6i = const.tile([6, 1], I32, tag="iota6i")
            nc.gpsimd.iota(iota6i[:, :], pattern=[[0, 1]], base=0,
                           channel_multiplier=1)
            iota6f = const.tile([6, 1], F32, tag="iota6f")
            nc.vector.tensor_copy(iota6f[:, :], iota6i[:, :])
            loglik = const.tile([1, NB], F32, tag="loglik")
            nc.vector.memset(loglik[:, :], 0.0)

            # ---------------- emission production ----------------
            em_tiles = [None, None]   # ping-pong of [em0,em1,em2]

            def produce_emissions(w):
                obs_i = obs_pool.tile([1, cols], I32, tag="obs_i")
                nc.sync.dma_start(
                    obs_i[0:1, :],
                    obsT[w * WIN:(w + 1) * WIN, :])
                obs_bf = obs_pool.tile([1, cols], BF16, tag="obs_bf")
                nc.vector.tensor_copy(obs_bf[:1, :], obs_i[:1, :])
                ohs = []
                for h in range(nh):
                    pobs = ps_e.tile([6, 512], F32, tag="pe")
                    nc.tensor.matmul(
                        pobs[:, :], ones6[0:1, :],
                        obs_bf[0:1, h * 512:(h + 1) * 512],
                        start=True, stop=True)
                    oh = oh_pool.tile([6, 512], BF16, tag="oh")
                    nc.vector.tensor_scalar(
                        oh[:, :], pobs[:, :], iota6f[:, 0:1], None,
                        op0=AluOpType.is_equal)
                    ohs.append(oh)
                ems = []
                for b in range(3):
                    emt = em_pools[b].tile([128, cols], BF16, tag=f"emt{b}")
                    for h in range(nh):
                        pe_ = ps_e.tile([128, 512], F32, tag="pe")
                        nc.tensor.matmul(
                            pe_[:, :], BmT[:, b * 128:(b + 1) * 128],
                            ohs[h][:, :], start=True, stop=True)
                        nc.scalar.activation(
                            emt[:, h * 512:(h + 1) * 512], pe_[:, :],
                            mybir.ActivationFunctionType.Copy)
                    ems.append(emt)
                em_tiles[w % 2] = ems

            produce_emissions(0)

            # ---------------- scan ----------------
            # alpha lives in 3 separate per-block SBUF tiles so the RAW
            # tracking is per slice: step t+1's k-chunk-j matmuls wait only
            # on DVE mult j of step t, not on the last of the three.
            # All scan tiles are STATIC ping-pong pairs (allocated once, not
            # per step) so Tile emits no per-step pool release bookkeeping.
            ADEPTH = 8   # alpha rotation depth: deep enough that the WAW
                         # self-wait on tile reuse is already satisfied and
                         # Tile skips emitting it
            ps_sets = [[], []]
            al_sets = [[] for _ in range(ADEPTH)]
            for p in (0, 1):
                for b in range(3):
                    ps_stat = ps_sl[b].tile([128, 16], F32, tag=f"psst{b}_{p}")
                    ps_sets[p].append(ps_stat)
            for p in range(ADEPTH):
                for b in range(3):
                    al_stat = alpha_pool[b].tile([128, 16], BF16,
                                                 tag=f"alst{b}_{p}")
                    al_sets[p].append(al_stat)
            avers = 0    # rotation counter over alpha writes

            alphas = None
            for w in range(nwin):
                ems = em_tiles[w % 2]
                for s_ in range(WIN):
                    t = w * WIN + s_
                    if t == 0:
                        alphas = al_sets[avers % ADEPTH]
                        avers += 1
                        for b in range(3):
                            nc.vector.tensor_scalar_mul(
                                alphas[b][:, :], ems[b][:, 0:16],
                                I_col[:, b:b + 1])
                        continue
                    # b-outer / k-inner: slice b completes at MM slot 3b+2,
                    # so DVE mult b (its own PSUM bank) runs while later
                    # slices still accumulate, and next step's k-chunk-b
                    # matmuls get alpha slice b with maximal lead time.
                    pslice = ps_sets[t % 2]
                    a_next = al_sets[avers % ADEPTH]
                    avers += 1
                    for b in range(3):
                        for k in range(3):
                            Kp = PBLK[k]
                            nc.tensor.matmul(
                                pslice[b][:, 0:16],
                                Abf[k][:Kp, b * 128:(b + 1) * 128],
                                alphas[k][:Kp, :],
                                start=(k == 0), stop=(k == 2))
                        nc.vector.tensor_tensor(
                            a_next[b][:, :], pslice[b][:, 0:16],
                            ems[b][:, s_ * 16:(s_ + 1) * 16],
                            op=AluOpType.mult)
                    alphas = a_next
                    # next window's emissions produced mid-window in a burst
                    if s_ == 8 and w + 1 < nwin:
                        produce_emissions(w + 1)

                # ---- renormalization: the running scale drifts only
                # ~0.07 log-units/step (the factor 6 folded into emissions
                # cancels the mean), so an exact rescale every 8 windows
                # (512 steps) keeps everything comfortably in range; the
                # log-sum telescopes across windows so loglik only needs
                # ln(z) at the points where we actually rescale.
                if (w + 1) % 2 != 0 and w + 1 < nwin:
                    continue
                pz = ps_e.tile([1, 16], F32, tag="pe")
                for k in range(3):
                    nc.tensor.matmul(
                        pz[:, :], onesz[:, 0:1], alphas[k][:, :],
                        start=(k == 0), stop=(k == 2))
                lnz = small.tile([1, 16], F32, tag="lnz")
                nc.scalar.activation(
                    lnz[:1, :], pz[:, :], mybir.ActivationFunctionType.Ln)
                nc.vector.tensor_add(loglik[:1, :], loglik[:1, :], lnz[:1, :])
                if w + 1 < nwin:
                    rz = small.tile([1, 16], F32, tag="rz")
                    nc.vector.reciprocal(rz[:1, :], pz[:, :])
                    prb = ps_e.tile([128, 16], F32, tag="pe")
                    nc.tensor.matmul(
                        prb[:, :], onesb[0:1, :], rz[0:1, :],
                        start=True, stop=True)
                    a_next = al_sets[avers % ADEPTH]
                    avers += 1
                    for b in range(3):
                        nc.vector.tensor_tensor(
                            a_next[b][:, :], alphas[b][:, :], prb[:, :],
                            op=AluOpType.mult)
                    alphas = a_next

            # ---------------- output ----------------
            outsb = small.tile([1, NB], F32, tag="outsb")
            nc.vector.tensor_scalar_add(
                outsb[:1, :], loglik[:1, :], -float(T) * math.log(CSCALE))
            nc.sync.dma_start(out_d[0:1, :], outsb[:1, :])

    return nc


def strip_alpha_selfwaits(nc):
    """Remove DVE self-waits on the scan mults' alpha-tile WAW.

    Tile emits a conservative same-engine wait when a mult rewrites an
    alpha ring slot (previous writer = the same engine 3*ADEPTH ticks
    earlier). The DVE executes its queue strictly in order, so the WAW is
    already ordered, and the stale-reader WAR is dominated by the mult's
    own PE wait (current step's matmuls are later in PE order than any
    old reader). Dropping the wait removes one EventSemaphore per scan
    step from the DVE queue, which sits on the step critical path.
    """
    for blk in nc.main_func.blocks:
        for ins in blk.instructions:
            if type(ins).__name__ not in ("InstTensorTensor",
                                          "InstTensorScalarPtr"):
                continue
            outs = getattr(ins, "outs", None)
            if not outs:
                continue
            memref = getattr(outs[0], "memref", "") or ""
            if not memref.startswith("al_stat"):
                continue
            si = ins.sync_info
            if si is None or not si.on_wait:
                continue
            keep = [w for w in si.on_wait if not w.ant_name.startswith("DVE")]
            if len(keep) != len(si.on_wait):
                si.on_wait = keep


def kernel(obs, I_kernel, A_kernel, B_kernel):
    # Tile's scheduler is sensitive to python str-hash ordering: with a
    # random hash seed the schedule is bimodal (measured 3.13 ms vs 3.75 ms
    # for identical code). Pin the good schedule by rerunning the whole
    # build+execute in a PYTHONHASHSEED=0 subprocess when needed.
    import os
    if os.environ.get("PYTHONHASHSEED") != "0":
        import subprocess
        import tempfile
        with tempfile.TemporaryDirectory() as td:
            fin = os.path.join(td, "in.npz")
            fout = os.path.join(td, "out.npy")
            np.savez(fin, obs=np.asarray(obs), I_kernel=np.asarray(I_kernel),
                     A_kernel=np.asarray(A_kernel),
                     B_kernel=np.asarray(B_kernel))
            env = dict(os.environ, PYTHONHASHSEED="0")
            subprocess.run(
                [sys.executable, os.path.abspath(__file__),
                 "--subprocess", fin, fout],
                env=env, check=True)
            global LAST_EXEC_NS
            try:
                with open(fout + ".time") as tf:
                    LAST_EXEC_NS = int(tf.read().strip() or 0) or None
            except OSError:
                LAST_EXEC_NS = None
            return np.load(fout)
    return _kernel_impl(obs, I_kernel, A_kernel, B_kernel)


def _kernel_impl(obs, I_kernel, A_kernel, B_kernel):
    obs = np.asarray(obs)
    T = obs.shape[1]
    nc = build(T=T)
    strip_alpha_selfwaits(nc)
    nc.finalize()
    A = np.ascontiguousarray(np.asarray(A_kernel, dtype=np.float32))
    B = np.ascontiguousarray(np.asarray(B_kernel, dtype=np.float32))
    I = np.ascontiguousarray(
        np.asarray(I_kernel, dtype=np.float32).reshape(1, S))
    in_maps = []
    for c in range(NCORES):
        shard = obs[c * NB:(c + 1) * NB]            # [16, T]
        in_maps.append({
            "obsT": np.ascontiguousarray(shard.T.astype(np.int32)),
            "A_kern": A,
            "B_kern": B,
            "I_kern": I,
        })
    res = run_bass_kernel_spmd(nc, in_maps, core_ids=list(range(NCORES)))
    global LAST_RESULTS
    LAST_RESULTS = res
    out = np.concatenate([r["out"].reshape(NB) for r in res.results])
    return out.astype(np.float32)


LAST_RESULTS = None
LAST_EXEC_NS = None


if __name__ == "__main__":
    if len(sys.argv) == 4 and sys.argv[1] == "--subprocess":
        import os as _os
        if _os.environ.get("BASS_TRACE") == "1":
            # best-effort: install the real NTFF hook so the subprocess
            # captures exec_time_ns (mirrors test.py's setup)
            try:
                sys.path.insert(0, "/root/.axon_site")
                from trn_agent_boot.trn_boot import _ntff_profile_via_ctypes
                import antenv.axon_hooks as _ah
                _ah.set_axon_ntff_profile_hook(
                    _ntff_profile_via_ctypes("/opt/axon/libaxon_pjrt.so"))
                import concourse.bass_utils as _bu
                _bu.upload_artifacts = lambda tmpdir: f"local://{tmpdir}"
            except Exception:
                pass
        data = np.load(sys.argv[2])
        out = _kernel_impl(data["obs"], data["I_kernel"],
                           data["A_kernel"], data["B_kernel"])
        np.save(sys.argv[3], out)
        et = LAST_RESULTS.exec_time_ns if LAST_RESULTS is not None else None
        with open(sys.argv[3] + ".time", "w") as tf:
            tf.write(str(et or 0))
        sys.exit(0)
    # smoke test with small T
    rng = np.random.default_rng(0)
    obs = rng.integers(0, EM, (128, 128), dtype=np.int32)
    I = rng.standard_normal(S).astype(np.float32)
    A = rng.standard_normal((S, S)).astype(np.float32)
    B = rng.standard_normal((S, EM)).astype(np.float32)
    print(kernel(obs, I, A, B)[:4])


# revision 61
# speedup vs baseline: 1.0004x; 1.0004x over previous
"""HMM forward log-likelihood (CgpHmmLayer) on 8 TRN2 NeuronCores.

Data-parallel over batch: 128 sequences -> 16 per core. Each core runs the
full T=4096 alpha scan for its shard with A/B/I kernels replicated; the
per-core [16] loglik outputs are concatenated on the host.

Layout: alpha is kept transposed ([state, seq]) with the 308 states split
into 3 partition blocks (128/128/52, padded), one SBUF tile per block so
dependency tracking is per-slice. Per scan step: 9 matmuls (3 k-chunks x 3
m-blocks; the A-tile is the stationary operand, alpha the N=16 moving one)
accumulate each m-block into its own PSUM bank (start=True zeroes a whole
bank, so concurrently-accumulating slices must not share one); 3 DVE
multiplies then apply the per-(state,seq) emission and write the next alpha
slices, letting step t+1's k-chunk-j matmuls start as soon as slice j is
ready. Emissions for window w+1 are built during window w via one-hot
matmuls (obs -> onehot[6,cols] -> Bm.T @ onehot). A factor of 6 is folded
into the emission matrix so the running scale stays O(1) (uniform obs make
E[z]=1/6 exactly); an exact renormalization (sum via ones-matmul, ln
accumulated into the telescoped loglik, reciprocal broadcast via a K=1
matmul) runs every 4 windows. All softmaxes run on device.

Measured on trn2.8x1: 3.13 ms HW exec, rel err 6.9e-6 vs the fp32 jax
reference (~700 ns/scan-step; critical path = matmul drain 190 + sem 38 +
DVE emission-mult 175 + sem 54 + ~240 of data/weight-paced streaming, plus
~80 ns/step of emission-precompute amortization). The kernel re-executes
itself in a PYTHONHASHSEED=0 subprocess because Tile's schedule is
hash-order sensitive (3.13 vs 3.75 ms bimodal otherwise).
"""

import sys
import types

sys.path.insert(0, "/opt/trn_rl_repo")

import math

import numpy as np

# If BASS_TRACE is set but this image's antenv lacks axon_hooks, the trace
# path of run_bass_kernel_spmd would die on import. Pre-install a stub that
# reports "no hook" so tracing degrades gracefully instead. A real hook
# installed earlier (e.g. by test.py) is left untouched.
try:
    from antenv.axon_hooks import get_axon_ntff_profile_hook  # noqa: F401
except ImportError:
    _mod = types.ModuleType("antenv.axon_hooks")
    _mod._hook = None
    _mod.get_axon_ntff_profile_hook = lambda: _mod._hook
    _mod.set_axon_ntff_profile_hook = lambda h: setattr(_mod, "_hook", h)
    sys.modules["antenv.axon_hooks"] = _mod

import concourse.bass as bass
import concourse.bacc as bacc
import concourse.mybir as mybir
from concourse.alu_op_type import AluOpType
from concourse.bass_utils import run_bass_kernel_spmd
from concourse.tile import TileContext

F32 = mybir.dt.float32
BF16 = mybir.dt.bfloat16
I32 = mybir.dt.int32

S, EM = 308, 6
NB = 16          # sequences per core
NCORES = 8
CSCALE = 6.0     # folded into emission probs to keep running scale ~O(1)


def build(T=4096, WIN=256):
    assert T % WIN == 0
    nwin = T // WIN
    cols = WIN * NB          # onehot columns per window
    assert cols % 512 == 0
    nh = cols // 512

    nc = bacc.Bacc("TRN2", target_bir_lowering=False, debug=False)
    obsT = nc.declare_dram_parameter("obsT", [T, NB], I32, isOutput=False)
    A_k = nc.declare_dram_parameter("A_kern", [S, S], F32, isOutput=False)
    B_k = nc.declare_dram_parameter("B_kern", [S, EM], F32, isOutput=False)
    I_k = nc.declare_dram_parameter("I_kern", [1, S], F32, isOutput=False)
    out_d = nc.declare_dram_parameter("out", [1, NB], F32, isOutput=True)
    scrB = nc.dram_tensor("scrB", [S, EM], BF16)
    scrI = nc.dram_tensor("scrI", [1, S], F32)

    PBLK = [128, 128, 52]   # rows of each state block

    with TileContext(nc) as tc:
        import contextlib
        ctx = contextlib.ExitStack()
        with ctx:
            const = ctx.enter_context(tc.tile_pool(name="const", bufs=1))
            work = ctx.enter_context(tc.tile_pool(name="work", bufs=2))
            obs_pool = ctx.enter_context(tc.tile_pool(name="obsp", bufs=2))
            oh_pool = ctx.enter_context(tc.tile_pool(name="ohp", bufs=9))
            em_pools = [
                ctx.enter_context(tc.tile_pool(name=f"em{b}", bufs=2))
                for b in range(3)
            ]
            alpha_pool = [
                ctx.enter_context(tc.tile_pool(name=f"alphap{b}", bufs=1))
                for b in range(3)
            ]
            small = ctx.enter_context(tc.tile_pool(name="small", bufs=2))
            # PSUM budget (8 banks): 3 scan-slice pools x 2 static
            # ping-pong tiles (each slice accumulates in its own bank since
            # start=True zeroes a whole bank) = 6, plus 2 banks shared by
            # the emission/onehot/normalization matmul outputs.
            ps_sl = [
                ctx.enter_context(
                    tc.tile_pool(name=f"ps_s{b}", bufs=1, space="PSUM"))
                for b in range(3)
            ]
            ps_e = ctx.enter_context(tc.tile_pool(name="ps_e", bufs=2, space="PSUM"))

            # ---------------- constants / preprocessing ----------------
            # A row-softmax -> 3 k-blocks [128, 384] bf16 (zero padded)
            Abf = []
            for k in range(3):
                P = PBLK[k]
                araw = work.tile([128, S], F32, tag="araw")
                nc.sync.dma_start(araw[:P, :], A_k[128 * k:128 * k + P, :])
                negmax = work.tile([128, 1], F32, tag="negmax")
                nc.vector.tensor_reduce(
                    negmax[:P], araw[:P, :], axis=mybir.AxisListType.X,
                    op=AluOpType.max, negate=True)
                ex = work.tile([128, S], F32, tag="ex")
                nc.scalar.activation(
                    ex[:P, :], araw[:P, :], mybir.ActivationFunctionType.Exp,
                    bias=negmax[:P])
                ssum = work.tile([128, 1], F32, tag="ssum")
                nc.vector.tensor_reduce(
                    ssum[:P], ex[:P, :], axis=mybir.AxisListType.X,
                    op=AluOpType.add)
                rs = work.tile([128, 1], F32, tag="rs")
                nc.vector.reciprocal(rs[:P], ssum[:P])
                At = const.tile([128, 384], BF16, tag=f"A{k}")
                nc.vector.memset(At[:, :], 0.0)
                nc.vector.tensor_scalar_mul(At[:P, 0:S], ex[:P, :], rs[:P])
                Abf.append(At)

            # B row-softmax * CSCALE -> scratch DRAM -> transposed BmT [6,384]
            for k in range(3):
                P = PBLK[k]
                braw = work.tile([128, EM], F32, tag="braw")
                nc.sync.dma_start(braw[:P, :], B_k[128 * k:128 * k + P, :])
                negmax = work.tile([128, 1], F32, tag="negmax")
                nc.vector.tensor_reduce(
                    negmax[:P], braw[:P, :], axis=mybir.AxisListType.X,
                    op=AluOpType.max, negate=True)
                ex = work.tile([128, EM], F32, tag="exb")
                nc.scalar.activation(
                    ex[:P, :], braw[:P, :], mybir.ActivationFunctionType.Exp,
                    bias=negmax[:P])
                ssum = work.tile([128, 1], F32, tag="ssum")
                nc.vector.tensor_reduce(
                    ssum[:P], ex[:P, :], axis=mybir.AxisListType.X,
                    op=AluOpType.add)
                rs = work.tile([128, 1], F32, tag="rs")
                nc.vector.reciprocal(rs[:P], ssum[:P])
                bmn = work.tile([128, EM], BF16, tag="bmn")
                nc.vector.tensor_scalar(
                    bmn[:P, :], ex[:P, :], rs[:P], CSCALE,
                    op0=AluOpType.mult, op1=AluOpType.mult)
                nc.sync.dma_start(scrB[128 * k:128 * k + P, :], bmn[:P, :])
            BmT = const.tile([6, 384], BF16, tag="BmT")
            nc.vector.memset(BmT[:, :], 0.0)
            for k in range(3):
                P = PBLK[k]
                nc.sync.dma_start(
                    BmT[:, 128 * k:128 * k + P],
                    scrB[128 * k:128 * k + P, :].rearrange("a b -> b a"))

            # I softmax -> scratch -> I_col [128, 3] f32 (zero padded)
            iraw = work.tile([1, S], F32, tag="iraw")
            nc.sync.dma_start(iraw[:1, :], I_k[0:1, :])
            inegmax = work.tile([1, 1], F32, tag="inegmax")
            nc.vector.tensor_reduce(
                inegmax[:1], iraw[:1, :], axis=mybir.AxisListType.X,
                op=AluOpType.max, negate=True)
            iex = work.tile([1, S], F32, tag="iex")
            nc.scalar.activation(
                iex[:1, :], iraw[:1, :], mybir.ActivationFunctionType.Exp,
                bias=inegmax[:1])
            issum = work.tile([1, 1], F32, tag="issum")
            nc.vector.tensor_reduce(
                issum[:1], iex[:1, :], axis=mybir.AxisListType.X,
                op=AluOpType.add)
            irs = work.tile([1, 1], F32, tag="irs")
            nc.vector.reciprocal(irs[:1], issum[:1])
            ism = work.tile([1, S], F32, tag="ism")
            nc.vector.tensor_scalar_mul(ism[:1, :], iex[:1, :], irs[:1])
            nc.sync.dma_start(scrI[0:1, :], ism[:1, :])
            I_col = const.tile([128, 3], F32, tag="I_col")
            nc.vector.memset(I_col[:, :], 0.0)
            for k in range(3):
                P = PBLK[k]
                nc.sync.dma_start(
                    I_col[:P, k:k + 1],
                    scrI[0:1, 128 * k:128 * k + P].rearrange("a b -> b a"))

            ones6 = const.tile([1, 6], BF16, tag="ones6")
            nc.vector.memset(ones6[:, :], 1.0)
            onesz = const.tile([128, 1], BF16, tag="onesz")
            nc.vector.memset(onesz[:, :], 1.0)
            onesb = const.tile([1, 128], F32, tag="onesb")
            nc.vector.memset(onesb[:, :], 1.0)
            iota6i = const.tile([6, 1], I32, tag="iota6i")
            nc.gpsimd.iota(iota6i[:, :], pattern=[[0, 1]], base=0,
                           channel_multiplier=1)
            iota6f = const.tile([6, 1], F32, tag="iota6f")
            nc.vector.tensor_copy(iota6f[:, :], iota6i[:, :])
            loglik = const.tile([1, NB], F32, tag="loglik")
            nc.vector.memset(loglik[:, :], 0.0)

            # ---------------- emission production ----------------
            em_tiles = [None, None]   # ping-pong of [em0,em1,em2]

            def produce_emissions(w):
                obs_i = obs_pool.tile([1, cols], I32, tag="obs_i")
                nc.sync.dma_start(
                    obs_i[0:1, :],
                    obsT[w * WIN:(w + 1) * WIN, :])
                obs_bf = obs_pool.tile([1, cols], BF16, tag="obs_bf")
                nc.vector.tensor_copy(obs_bf[:1, :], obs_i[:1, :])
                ohs = []
                for h in range(nh):
                    pobs = ps_e.tile([6, 512], F32, tag="pe")
                    nc.tensor.matmul(
                        pobs[:, :], ones6[0:1, :],
                        obs_bf[0:1, h * 512:(h + 1) * 512],
                        start=True, stop=True)
                    oh = oh_pool.tile([6, 512], BF16, tag="oh")
                    nc.vector.tensor_scalar(
                        oh[:, :], pobs[:, :], iota6f[:, 0:1], None,
                        op0=AluOpType.is_equal)
                    ohs.append(oh)
                ems = []
                for b in range(3):
                    emt = em_pools[b].tile([128, cols], BF16, tag=f"emt{b}")
                    for h in range(nh):
                        pe_ = ps_e.tile([128, 512], F32, tag="pe")
                        nc.tensor.matmul(
                            pe_[:, :], BmT[:, b * 128:(b + 1) * 128],
                            ohs[h][:, :], start=True, stop=True)
                        nc.scalar.activation(
                            emt[:, h * 512:(h + 1) * 512], pe_[:, :],
                            mybir.ActivationFunctionType.Copy)
                    ems.append(emt)
                em_tiles[w % 2] = ems

            produce_emissions(0)

            # ---------------- scan ----------------
            # alpha lives in 3 separate per-block SBUF tiles so the RAW
            # tracking is per slice: step t+1's k-chunk-j matmuls wait only
            # on DVE mult j of step t, not on the last of the three.
            # All scan tiles are STATIC ping-pong pairs (allocated once, not
            # per step) so Tile emits no per-step pool release bookkeeping.
            ADEPTH = 8   # alpha rotation depth: deep enough that the WAW
                         # self-wait on tile reuse is already satisfied and
                         # Tile skips emitting it
            ps_sets = [[], []]
            al_sets = [[] for _ in range(ADEPTH)]
            for p in (0, 1):
                for b in range(3):
                    ps_stat = ps_sl[b].tile([128, 16], F32, tag=f"psst{b}_{p}")
                    ps_sets[p].append(ps_stat)
            for p in range(ADEPTH):
                for b in range(3):
                    al_stat = alpha_pool[b].tile([128, 16], BF16,
                                                 tag=f"alst{b}_{p}")
                    al_sets[p].append(al_stat)
            avers = 0    # rotation counter over alpha writes

            alphas = None
            for w in range(nwin):
                ems = em_tiles[w % 2]
                for s_ in range(WIN):
                    t = w * WIN + s_
                    if t == 0:
                        alphas = al_sets[avers % ADEPTH]
                        avers += 1
                        for b in range(3):
                            nc.vector.tensor_scalar_mul(
                                alphas[b][:, :], ems[b][:, 0:16],
                                I_col[:, b:b + 1])
                        continue
                    # b-outer / k-inner: slice b completes at MM slot 3b+2,
                    # so DVE mult b (its own PSUM bank) runs while later
                    # slices still accumulate, and next step's k-chunk-b
                    # matmuls get alpha slice b with maximal lead time.
                    pslice = ps_sets[t % 2]
                    a_next = al_sets[avers % ADEPTH]
                    avers += 1
                    for b in range(3):
                        for k in range(3):
                            Kp = PBLK[k]
                            nc.tensor.matmul(
                                pslice[b][:, 0:16],
                                Abf[k][:Kp, b * 128:(b + 1) * 128],
                                alphas[k][:Kp, :],
                                start=(k == 0), stop=(k == 2))
                        nc.vector.tensor_tensor(
                            a_next[b][:, :], pslice[b][:, 0:16],
                            ems[b][:, s_ * 16:(s_ + 1) * 16],
                            op=AluOpType.mult)
                    alphas = a_next
                    # next window's emissions produced mid-window in a burst
                    if s_ == 8 and w + 1 < nwin:
                        produce_emissions(w + 1)

                # ---- renormalization: the running scale drifts only
                # ~0.07 log-units/step (the factor 6 folded into emissions
                # cancels the mean), so an exact rescale every 8 windows
                # (512 steps) keeps everything comfortably in range; the
                # log-sum telescopes across windows so loglik only needs
                # ln(z) at the points where we actually rescale.
                if (w + 1) % 2 != 0 and w + 1 < nwin:
                    continue
                pz = ps_e.tile([1, 16], F32, tag="pe")
                for k in range(3):
                    nc.tensor.matmul(
                        pz[:, :], onesz[:, 0:1], alphas[k][:, :],
                        start=(k == 0), stop=(k == 2))
                lnz = small.tile([1, 16], F32, tag="lnz")
                nc.scalar.activation(
                    lnz[:1, :], pz[:, :], mybir.ActivationFunctionType.Ln)
                nc.vector.tensor_add(loglik[:1, :], loglik[:1, :], lnz[:1, :])
                if w + 1 < nwin:
                    rz = small.tile([1, 16], F32, tag="rz")
                    nc.vector.reciprocal(rz[:1, :], pz[:, :])
                    prb = ps_e.tile([128, 16], F32, tag="pe")
                    nc.tensor.matmul(
                        prb[:, :], onesb[0:1, :], rz[0:1, :],
                        start=True, stop=True)
                    a_next = al_sets[avers % ADEPTH]
                    avers += 1
                    for b in range(3):
                        nc.vector.tensor_tensor(
                            a_next[b][:, :], alphas[b][:, :], prb[:, :],
                            op=AluOpType.mult)
                    alphas = a_next

            # ---------------- output ----------------
            outsb = small.tile([1, NB], F32, tag="outsb")
            nc.vector.tensor_scalar_add(
                outsb[:1, :], loglik[:1, :], -float(T) * math.log(CSCALE))
            nc.sync.dma_start(out_d[0:1, :], outsb[:1, :])

    return nc


def strip_alpha_selfwaits(nc):
    """Remove DVE self-waits on the scan mults' alpha-tile WAW.

    Tile emits a conservative same-engine wait when a mult rewrites an
    alpha ring slot (previous writer = the same engine 3*ADEPTH ticks
    earlier). The DVE executes its queue strictly in order, so the WAW is
    already ordered, and the stale-reader WAR is dominated by the mult's
    own PE wait (current step's matmuls are later in PE order than any
    old reader). Dropping the wait removes one EventSemaphore per scan
    step from the DVE queue, which sits on the step critical path.
    """
    for blk in nc.main_func.blocks:
        for ins in blk.instructions:
            if type(ins).__name__ not in ("InstTensorTensor",
                                          "InstTensorScalarPtr"):
                continue
            outs = getattr(ins, "outs", None)
            if not outs:
                continue
            memref = getattr(outs[0], "memref", "") or ""
            if not memref.startswith("al_stat"):
                continue
            si = ins.sync_info
            if si is None or not si.on_wait:
                continue
            keep = [w for w in si.on_wait if not w.ant_name.startswith("DVE")]
            if len(keep) != len(si.on_wait):
                si.on_wait = keep


def kernel(obs, I_kernel, A_kernel, B_kernel):
    # Tile's scheduler is sensitive to python str-hash ordering: with a
    # random hash seed the schedule is bimodal (measured 3.13 ms vs 3.75 ms
    # for identical code). Pin the good schedule by rerunning the whole
    # build+execute in a PYTHONHASHSEED=0 subprocess when needed.
    import os
    if os.environ.get("PYTHONHASHSEED") != "0":
        import subprocess
        import tempfile
        with tempfile.TemporaryDirectory() as td:
            fin = os.path.join(td, "in.npz")
            fout = os.path.join(td, "out.npy")
            np.savez(fin, obs=np.asarray(obs), I_kernel=np.asarray(I_kernel),
                     A_kernel=np.asarray(A_kernel),
                     B_kernel=np.asarray(B_kernel))
            env = dict(os.environ, PYTHONHASHSEED="0")
            subprocess.run(
                [sys.executable, os.path.abspath(__file__),
                 "--subprocess", fin, fout],
                env=env, check=True)
            global LAST_EXEC_NS
            try:
                with open(fout + ".time") as tf:
                    LAST_EXEC_NS = int(tf.read().strip() or 0) or None
            except OSError:
                LAST_EXEC_NS = None
            return np.load(fout)
    return _kernel_impl(obs, I_kernel, A_kernel, B_kernel)


def _kernel_impl(obs, I_kernel, A_kernel, B_kernel):
    obs = np.asarray(obs)
    T = obs.shape[1]
    nc = build(T=T)
    strip_alpha_selfwaits(nc)
    nc.finalize()
    A = np.ascontiguousarray(np.asarray(A_kernel, dtype=np.float32))
    B = np.ascontiguousarray(np.asarray(B_kernel, dtype=np.float32))
    I = np.ascontiguousarray(
        np.asarray(I_kernel, dtype=np.float32).reshape(1, S))
    in_maps = []
    for c in range(NCORES):
        shard = obs[c * NB:(c + 1) * NB]            # [16, T]
        in_maps.append({
            "obsT": np.ascontiguousarray(shard.T.astype(np.int32)),
            "A_kern": A,
            "B_kern": B,
            "I_kern": I,
        })
    res = run_bass_kernel_spmd(nc, in_maps, core_ids=list(range(NCORES)))
    global LAST_RESULTS
    LAST_RESULTS = res
    out = np.concatenate([r["out"].reshape(NB) for r in res.results])
    return out.astype(np.float32)


LAST_RESULTS = None
LAST_EXEC_NS = None


if __name__ == "__main__":
    if len(sys.argv) == 4 and sys.argv[1] == "--subprocess":
        import os as _os
        if _os.environ.get("BASS_TRACE") == "1":
            # best-effort: install the real NTFF hook so the subprocess
            # captures exec_time_ns (mirrors test.py's setup)
            try:
                sys.path.insert(0, "/root/.axon_site")
                from trn_agent_boot.trn_boot import _ntff_profile_via_ctypes
                import antenv.axon_hooks as _ah
                _ah.set_axon_ntff_profile_hook(
                    _ntff_profile_via_ctypes("/opt/axon/libaxon_pjrt.so"))
                import concourse.bass_utils as _bu
                _bu.upload_artifacts = lambda tmpdir: f"local://{tmpdir}"
            except Exception:
                pass
        data = np.load(sys.argv[2])
        out = _kernel_impl(data["obs"], data["I_kernel"],
                           data["A_kernel"], data["B_kernel"])
        np.save(sys.argv[3], out)
        et = LAST_RESULTS.exec_time_ns if LAST_RESULTS is not None else None
        with open(sys.argv[3] + ".time", "w") as tf:
            tf.write(str(et or 0))
        sys.exit(0)
    # smoke test with small T
    rng = np.random.default_rng(0)
    obs = rng.integers(0, EM, (128, 128), dtype=np.int32)
    I = rng.standard_normal(S).astype(np.float32)
    A = rng.standard_normal((S, S)).astype(np.float32)
    B = rng.standard_normal((S, EM)).astype(np.float32)
    print(kernel(obs, I, A, B)[:4])
